# revision 4
# baseline (speedup 1.0000x reference)
"""Trainium2 Bass kernel for nn_AnalysisConvolutionBlock.

Pipeline per sample: spectral-norm partial conv (3x3, 64->128) -> partial GDN
-> CDF 9/7 DWT (2D, stride 2, reflect pad). Data-parallel over batch: 8
samples -> 8 NeuronCores, no collectives.

Host precomputes (cheap, O(B*H*W) single-channel or O(C^2)): spectral norm
sigma, mask renormalization maps R/S, downsampled output masks, input packing
(x*mask, zero-padded, bf16). Device does all O(B*C*H*W) work.
"""
import os
import sys

import numpy as np

for _p in ("/opt/trn_rl_repo", "/root/.axon_site/_ro/trn_rl_repo"):
    if os.path.isdir(_p) and _p not in sys.path:
        sys.path.insert(0, _p)

import ml_dtypes

BF16 = ml_dtypes.bfloat16

# CDF 9/7 analysis filters (identical constants to the reference)
H0 = np.array([0.026748757410810, -0.016864118442875, -0.078223266528990,
               0.266864118442875, 0.602949018236360, 0.266864118442875,
               -0.078223266528990, -0.016864118442875, 0.026748757410810],
              np.float32)
H1 = np.array([0.091271763114250, -0.057543526228500, -0.591271763114250,
               1.115087052457000, -0.591271763114250, -0.057543526228500,
               0.091271763114250], np.float32)

B, CIN, COUT, H, W = 8, 64, 128, 256, 256
HO, WO = H // 2, W // 2
N_CORES = 8
STRIP1 = 16          # phase-1 output rows per strip
NS1 = H // STRIP1    # 16 strips
STRIP2 = 16          # phase-2 output (ho) rows per strip
NS2 = HO // STRIP2   # 8 strips

_BUILT = None        # (nc,) cache
LAST_RESULT = None   # BassKernelResults of the most recent run (for test.py)


def _refl(i, n):
    if i < 0:
        return -i
    if i >= n:
        return 2 * (n - 1) - i
    return i


def build_nc():
    import concourse.bass as bass
    import concourse.bacc as bacc
    import concourse.tile as tile
    from concourse import mybir

    f32 = mybir.dt.float32
    bf16 = mybir.dt.bfloat16
    AF = mybir.ActivationFunctionType
    OP = mybir.AluOpType

    nc = bacc.Bacc(None, target_bir_lowering=False)

    # ---- DRAM I/O (per-core shard shapes) ----
    xm_d = nc.declare_dram_parameter("xm", [CIN, H + 2, W + 2], bf16, isOutput=False)
    r_d = nc.declare_dram_parameter("rmap", [H, W], bf16, isOutput=False)
    s_d = nc.declare_dram_parameter("smap", [H, W], bf16, isOutput=False)
    wt_d = nc.declare_dram_parameter("wtaps", [CIN, 9 * COUT], bf16, isOutput=False)
    bias_d = nc.declare_dram_parameter("biasT", [1, COUT], bf16, isOutput=False)
    gT_d = nc.declare_dram_parameter("gammaT", [COUT, COUT], bf16, isOutput=False)
    beta_d = nc.declare_dram_parameter("beta", [COUT, 1], f32, isOutput=False)
    out_d = nc.declare_dram_parameter("out", [4, COUT, HO, WO], f32, isOutput=True)

    y2_d = nc.dram_tensor("y2tmp", [COUT, H, W], bf16)

    with tile.TileContext(nc) as tc:
        with tc.tile_pool(name="const", bufs=1) as const_pool:
            wt_sb = const_pool.tile([CIN, 9 * COUT], bf16)
            nc.sync.dma_start(out=wt_sb, in_=wt_d[:, :])
            bias_sb = const_pool.tile([1, COUT], bf16)
            nc.sync.dma_start(out=bias_sb, in_=bias_d[:, :])
            gT_sb = const_pool.tile([COUT, COUT], bf16)
            nc.sync.dma_start(out=gT_sb, in_=gT_d[:, :])
            beta_sb = const_pool.tile([COUT, 1], f32)
            nc.sync.dma_start(out=beta_sb, in_=beta_d[:, :])

            # ================= Phase 1: conv + GDN -> y2 =================
            with (
                tc.tile_pool(name="p1_in", bufs=3) as p1_in,
                tc.tile_pool(name="p1_tmp", bufs=3) as p1_tmp,
                tc.tile_pool(name="p1_out", bufs=2) as p1_out,
                tc.tile_pool(name="psq", bufs=2, space="PSUM") as psq,
                tc.tile_pool(name="psg", bufs=2, space="PSUM") as psg,
            ):
                for s in range(NS1):
                    h0 = s * STRIP1
                    xt = p1_in.tile([CIN, STRIP1 + 2, W + 2], bf16, tag="xt")
                    nc.sync.dma_start(out=xt, in_=xm_d[:, h0:h0 + STRIP1 + 2, :])
                    rb = p1_in.tile([128, STRIP1, W], bf16, tag="rb")
                    r_src = bass.AP(
                        tensor=r_d.tensor if hasattr(r_d, "tensor") else r_d,
                        offset=h0 * W,
                        ap=[[0, 128], [W, STRIP1], [1, W]],
                    )
                    nc.gpsimd.dma_start(out=rb, in_=r_src)
                    st = p1_in.tile([1, STRIP1, W], bf16, tag="st")
                    nc.sync.dma_start(out=st, in_=s_d[h0:h0 + STRIP1, :])

                    y2s = p1_out.tile([COUT, STRIP1, W], bf16, tag="y2s")

                    for t in range(STRIP1 // 2):  # 2 output rows per psum tile
                        r = 2 * t
                        q = psq.tile([COUT, 512], mybir.dt.float32, tag="q")
                        i = 0
                        for dy in range(3):
                            for dx in range(3):
                                rhs = xt[:, r + dy:r + dy + 2, dx:dx + W]
                                nc.tensor.matmul(
                                    q, wt_sb[:, i * COUT:(i + 1) * COUT], rhs,
                                    start=(i == 0), stop=False)
                                i += 1
                        nc.tensor.matmul(q, bias_sb, st[:, r:r + 2, :],
                                         start=False, stop=True)

                        y1 = p1_tmp.tile([COUT, 512], bf16, tag="y1")
                        nc.vector.tensor_tensor(
                            out=y1, in0=q, in1=rb[:, r:r + 2, :], op=OP.mult)
                        sq = p1_tmp.tile([COUT, 512], bf16, tag="sq")
                        nc.vector.tensor_tensor(out=sq, in0=y1, in1=y1, op=OP.mult)
                        g = psg.tile([COUT, 512], mybir.dt.float32, tag="g")
                        nc.tensor.matmul(g, gT_sb, sq, start=True, stop=True)
                        tln = p1_tmp.tile([COUT, 512], bf16, tag="tln")
                        nc.scalar.activation(out=tln, in_=g, func=AF.Ln,
                                             bias=beta_sb, scale=1.0)
                        inv = p1_tmp.tile([COUT, 512], bf16, tag="inv")
                        nc.scalar.activation(out=inv, in_=tln, func=AF.Exp,
                                             scale=-0.5)
                        nc.vector.tensor_tensor(
                            out=y2s[:, r:r + 2, :], in0=y1, in1=inv, op=OP.mult)

                    nc.sync.dma_start(out=y2_d[:, h0:h0 + STRIP1, :], in_=y2s)

            # ================= Phase 2: DWT =================
            with (
                tc.tile_pool(name="p2_in", bufs=2) as p2_in,
                tc.tile_pool(name="p2_mid", bufs=2) as p2_mid,
                tc.tile_pool(name="p2_out", bufs=2) as p2_out,
            ):
                NROWS = 2 * STRIP2 + 8  # 40 y2 rows per strip (with halo)
                for s in range(NS2):
                    ho0 = s * STRIP2
                    rlo = 2 * ho0 - 4
                    y2t = p2_in.tile([COUT, NROWS, W], bf16, tag="y2t")
                    # main load: valid rows [max(rlo,0), min(rlo+40, 256))
                    v0 = max(rlo, 0)
                    v1 = min(rlo + NROWS, H)
                    nc.sync.dma_start(out=y2t[:, v0 - rlo:v1 - rlo, :],
                                      in_=y2_d[:, v0:v1, :])
                    # reflected halo rows (first/last strip only)
                    for j in range(NROWS):
                        rr = rlo + j
                        if rr < 0 or rr >= H:
                            nc.sync.dma_start(
                                out=y2t[:, j:j + 1, :],
                                in_=y2_d[:, _refl(rr, H):_refl(rr, H) + 1, :])

                    # ---- H-stage: tile row for (j, k) = 2j + k_off ----
                    WP = W + 8  # lo_r/hi_r padded by 4 cols each side
                    lo_r = p2_mid.tile([COUT, STRIP2, WP], bf16, tag="lo_r")
                    hi_r = p2_mid.tile([COUT, STRIP2, WP], bf16, tag="hi_r")
                    eo = y2t.rearrange("p (j two) w -> p two j w", two=2)
                    for (dst, filt, off) in ((lo_r, H0, 0), (hi_r, H1, 1)):
                        for k in range(len(filt)):
                            row = k + off  # tile row offset: 2j + k + off
                            src = eo[:, row % 2, row // 2:row // 2 + STRIP2, :]
                            if k == 0:
                                nc.vector.tensor_scalar_mul(
                                    out=dst[:, :, 4:4 + W], in0=src,
                                    scalar1=float(filt[k]))
                            else:
                                nc.vector.scalar_tensor_tensor(
                                    out=dst[:, :, 4:4 + W], in0=src,
                                    scalar=float(filt[k]),
                                    in1=dst[:, :, 4:4 + W],
                                    op0=OP.mult, op1=OP.add)
                        # reflect-pad 4 columns each side:
                        # pad col p (0..3) <- data col 8-p ; col 260+p <- 258-p
                        for p in range(4):
                            nc.vector.tensor_copy(
                                out=dst[:, :, p:p + 1], in_=dst[:, :, 8 - p:9 - p])
                            nc.vector.tensor_copy(
                                out=dst[:, :, 260 + p:261 + p],
                                in_=dst[:, :, 258 - p:259 - p])

                    # ---- W-stage: out col wo reads padded col 2wo+k+off ----
                    outs = []
                    for oi, (srcb, filt, off) in enumerate(
                            ((lo_r, H0, 0), (lo_r, H1, 1),
                             (hi_r, H0, 0), (hi_r, H1, 1))):
                        ot = p2_out.tile([COUT, STRIP2, WO], mybir.dt.float32,
                                         tag=f"ot{oi}")
                        ceo = srcb.rearrange("p j (wo two) -> p j wo two", two=2)
                        eng = nc.vector
                        for k in range(len(filt)):
                            col = k + off
                            src = ceo[:, :, col // 2:col // 2 + WO, col % 2]
                            if k == 0:
                                eng.tensor_scalar_mul(out=ot, in0=src,
                                                      scalar1=float(filt[k]))
                            else:
                                eng.scalar_tensor_tensor(
                                    out=ot, in0=src, scalar=float(filt[k]),
                                    in1=ot, op0=OP.mult, op1=OP.add)
                        outs.append(ot)

                    for oi, ot in enumerate(outs):
                        nc.sync.dma_start(
                            out=out_d[oi, :, ho0:ho0 + STRIP2, :], in_=ot)

    nc.finalize()
    return nc


def _host_prep(tensor, mask, weight, bias, u, beta, gamma):
    f32 = np.float32
    O = weight.shape[0]
    wm = weight.reshape(O, -1).astype(f32)
    v = wm.T @ u.astype(f32)
    v = v / (np.linalg.norm(v) + 1e-12)
    sigma = np.linalg.norm(wm @ v)
    w_sn = (weight / sigma).astype(f32)

    m = mask[:, 0].astype(f32)  # [B,H,W]
    mp = np.pad(m, ((0, 0), (1, 1), (1, 1)))
    msum = np.zeros_like(m)
    for dy in range(3):
        for dx in range(3):
            msum += mp[:, dy:dy + H, dx:dx + W]
    valid = msum > 0
    R = np.where(valid, 9.0 / np.maximum(msum, 1e-8), 0.0).astype(f32)
    S = np.where(valid, msum / 9.0, 0.0).astype(f32)
    mval = valid.astype(f32)

    xm = (tensor * mask).astype(f32)
    xm_pad = np.zeros((B, CIN, H + 2, W + 2), BF16)
    xm_pad[:, :, 1:-1, 1:-1] = xm

    # wtaps[c, 9*o] with tap index i = 3*dy + dx : lhsT[c, o] = w_sn[o, c, dy, dx]
    wtaps = np.ascontiguousarray(
        w_sn.transpose(1, 2, 3, 0).reshape(CIN, 9 * COUT)).astype(BF16)
    biasT = np.ascontiguousarray(bias.astype(f32).reshape(1, COUT)).astype(BF16)
    gammaT = np.ascontiguousarray(gamma.astype(f32).T).astype(BF16)
    betac = np.ascontiguousarray(beta.astype(f32).reshape(COUT, 1))
    return w_sn, xm_pad, R.astype(BF16), S.astype(BF16), mval, wtaps, biasT, gammaT, betac


def _mask_down(m, K, axis):
    p = (K - 1) // 2
    pads = [(0, 0)] * m.ndim
    pads[axis] = (p, p)
    mp = np.pad(m, pads, mode='reflect')
    n_out = m.shape[axis] // 2
    shp = list(m.shape)
    shp[axis] = n_out
    out = np.zeros(shp, m.dtype)
    idx = [slice(None)] * m.ndim
    for k in range(K):
        ik = list(idx)
        ik[axis] = slice(k, k + 2 * n_out, 2)
        out = out + mp[tuple(ik)]
    return (out > 0).astype(m.dtype)


def kernel(tensor, mask, weight, bias, u, beta, gamma):
    global _BUILT, LAST_RESULT
    from concourse.bass_utils import run_bass_kernel_spmd

    tensor = np.asarray(tensor, np.float32)
    mask = np.asarray(mask, np.float32)

    (w_sn, xm_pad, Rb, Sb, mval, wtaps, biasT, gammaT, betac) = _host_prep(
        tensor, mask, np.asarray(weight), np.asarray(bias), np.asarray(u),
        np.asarray(beta), np.asarray(gamma))

    if _BUILT is None:
        _BUILT = build_nc()
    nc = _BUILT

    in_maps = []
    for b in range(N_CORES):
        in_maps.append({
            "xm": np.ascontiguousarray(xm_pad[b]),
            "rmap": np.ascontiguousarray(Rb[b]),
            "smap": np.ascontiguousarray(Sb[b]),
            "wtaps": wtaps,
            "biasT": biasT,
            "gammaT": gammaT,
            "beta": betac,
        })

    trace = bool(int(os.environ.get("BASS_KERNEL_TRACE", "0")))
    res = run_bass_kernel_spmd(nc, in_maps, core_ids=list(range(N_CORES)),
                               trace=trace)
    LAST_RESULT = res

    ll = np.stack([res.results[b]["out"][0] for b in range(N_CORES)])
    lh = np.stack([res.results[b]["out"][1] for b in range(N_CORES)])
    hl = np.stack([res.results[b]["out"][2] for b in range(N_CORES)])
    hh = np.stack([res.results[b]["out"][3] for b in range(N_CORES)])

    m4 = mval[:, None]  # [B,1,H,W]
    m_lo = _mask_down(m4, 9, 2)
    m_hi = _mask_down(m4, 7, 2)
    m_ll = _mask_down(m_lo, 9, 3)
    m_lh = _mask_down(m_lo, 7, 3)
    m_hl = _mask_down(m_hi, 9, 3)
    m_hh = _mask_down(m_hi, 7, 3)

    return (ll.astype(np.float32), m_ll, lh.astype(np.float32),
            hl.astype(np.float32), hh.astype(np.float32), m_lh, m_hl, m_hh)


# revision 18
# speedup vs baseline: 2.8348x; 2.8348x over previous
"""Trainium2 Bass kernel for nn_AnalysisConvolutionBlock.

Pipeline per sample: spectral-norm partial conv (3x3, 64->128) -> partial GDN
-> CDF 9/7 DWT (2D, stride 2, reflect pad). Data-parallel over batch: 8
samples -> 8 NeuronCores, no collectives.

Host precomputes (cheap, O(B*H*W) single-channel or O(C^2)): spectral norm
sigma, mask renormalization maps R/S, downsampled output masks, input packing
(x*mask, zero-padded, bf16), reflect-folded DWT filter matrices.

Device plan (per core / sample):
  Phase 1 (strips of 16 rows): conv as 7 accumulating bf16 matmuls per
  512-pixel tile (row-pair packing: dy=0,1 share K=128 matmuls), + K=1 bias
  matmul; y1 = psum * R (DVE); sq = y1^2 (gpsimd); G = gammaT @ sq (PE);
  inv = abs_rsqrt(G + beta) (ACT, single table set); y2 = y1 * inv (DVE).
  y2 written to DRAM in [w, c, h] layout (h contiguous) in 64-row chunks.
  Phase 2 (channel groups of 8): read y2 as [w-part, c, h] tiles; W-axis DWT
  as data-stationary matmuls (stationary = image block [w,h], moving =
  folded filter matrix) -> output lands transposed [h, wo] in PSUM; copy to
  SBUF bf16; H-axis DWT as regular matmuls with folded F_h. No transposes.
"""
import os
import sys

import numpy as np

for _p in ("/opt/trn_rl_repo", "/root/.axon_site/_ro/trn_rl_repo"):
    if os.path.isdir(_p) and _p not in sys.path:
        sys.path.insert(0, _p)

import ml_dtypes

BF16 = ml_dtypes.bfloat16

# CDF 9/7 analysis filters (identical constants to the reference)
H0 = np.array([0.026748757410810, -0.016864118442875, -0.078223266528990,
               0.266864118442875, 0.602949018236360, 0.266864118442875,
               -0.078223266528990, -0.016864118442875, 0.026748757410810],
              np.float32)
H1 = np.array([0.091271763114250, -0.057543526228500, -0.591271763114250,
               1.115087052457000, -0.591271763114250, -0.057543526228500,
               0.091271763114250], np.float32)

B, CIN, COUT, H, W = 8, 64, 128, 256, 256
HO, WO = H // 2, W // 2
N_CORES = 8
STRIP1 = 16          # phase-1 output rows per strip
NS1 = H // STRIP1    # 16 strips
CG = 8               # phase-2 channels per group
NG = COUT // CG      # 16 groups

_BUILT = None
LAST_RESULT = None   # BassKernelResults of the most recent run (for test.py)


def _refl(i, n):
    if i < 0:
        return -i
    if i >= n:
        return 2 * (n - 1) - i
    return i


def _fold_filter(filt, n):
    """[n//2, n] matrix F[o, i] = sum of filt taps hitting input i for output
    o (stride 2, reflect pad)."""
    K = len(filt)
    c0 = (K - 1) // 2
    F = np.zeros((n // 2, n), np.float32)
    for o in range(n // 2):
        for k in range(K):
            F[o, _refl(2 * o + k - c0, n)] += filt[k]
    return F


def build_nc():
    import concourse.bass as bass
    import concourse.bacc as bacc
    import concourse.tile as tile
    from concourse import mybir

    dbg_p2 = int(os.environ.get("BK_P2", "1"))
    dbg_scatter = int(os.environ.get("BK_SCATTER", "1"))
    dbg_gp = int(os.environ.get("BK_GPSIMD", "1"))
    dbg_p2stage = int(os.environ.get("BK_P2STAGE", "3"))

    f32 = mybir.dt.float32
    bf16 = mybir.dt.bfloat16
    AF = mybir.ActivationFunctionType
    OP = mybir.AluOpType

    nc = bacc.Bacc(None, target_bir_lowering=False)

    # ---- DRAM I/O (per-core shard shapes) ----
    xm_d = nc.declare_dram_parameter("xm", [CIN, H + 2, W + 2], bf16, isOutput=False)
    r_d = nc.declare_dram_parameter("rmap", [H, W], bf16, isOutput=False)
    s_d = nc.declare_dram_parameter("smap", [H, W], bf16, isOutput=False)
    wp_d = nc.declare_dram_parameter("wp", [128, 3 * COUT], bf16, isOutput=False)
    ws_d = nc.declare_dram_parameter("ws", [CIN, 3 * COUT], bf16, isOutput=False)
    bias_d = nc.declare_dram_parameter("biasT", [1, COUT], bf16, isOutput=False)
    gT_d = nc.declare_dram_parameter("gammaT", [COUT, COUT], bf16, isOutput=False)
    beta_d = nc.declare_dram_parameter("beta", [COUT, 1], f32, isOutput=False)
    # W-axis folded filters: fw[wh] = [F0w[:, wh].T | F1w[:, wh].T]  [128w, 256]
    fw_d = nc.declare_dram_parameter("fw", [2, 128, 256], bf16, isOutput=False)
    # H-axis folded filters: fh[f*2+hb] = F{f}h[:, hb].T  [128h, 128ho]
    fh_d = nc.declare_dram_parameter("fh", [4, 128, 128], bf16, isOutput=False)
    # out layout: [oi, ho, c, wo] (host transposes to [c, ho, wo])
    out_d = nc.declare_dram_parameter("out", [4, HO, COUT, WO], f32, isOutput=True)

    # y2 scratch in [w, c, h] layout (h contiguous)
    y2_d = nc.dram_tensor("y2tmp", [W, COUT, H], bf16)

    with tile.TileContext(nc) as tc:
        with tc.tile_pool(name="const", bufs=1) as cpool:
            wp_sb = cpool.tile([128, 3 * COUT], bf16)
            nc.sync.dma_start(out=wp_sb, in_=wp_d[:, :])
            ws_sb = cpool.tile([CIN, 3 * COUT], bf16)
            nc.sync.dma_start(out=ws_sb, in_=ws_d[:, :])
            bias_sb = cpool.tile([1, COUT], bf16)
            nc.sync.dma_start(out=bias_sb, in_=bias_d[:, :])
            gT_sb = cpool.tile([COUT, COUT], bf16)
            nc.sync.dma_start(out=gT_sb, in_=gT_d[:, :])
            beta_sb = cpool.tile([COUT, 1], f32)
            nc.sync.dma_start(out=beta_sb, in_=beta_d[:, :])
            fw_sb = cpool.tile([128, 2, 256], bf16)
            nc.sync.dma_start(out=fw_sb[:, 0, :], in_=fw_d[0])
            nc.sync.dma_start(out=fw_sb[:, 1, :], in_=fw_d[1])
            fh_sb = cpool.tile([128, 4, 128], bf16)
            for i in range(4):
                nc.sync.dma_start(out=fh_sb[:, i, :], in_=fh_d[i])

            # ================= Phase 1: conv + GDN -> y2 =================
            with (
                tc.tile_pool(name="p1_in", bufs=3) as p1_in,
                tc.tile_pool(name="p1_tmp", bufs=3) as p1_tmp,
                tc.tile_pool(name="p1_out", bufs=2) as p1_out,
                tc.tile_pool(name="psq", bufs=2, space="PSUM") as psq,
                tc.tile_pool(name="psg", bufs=2, space="PSUM") as psg,
            ):
                for sg in range(NS1 // 4):   # y2 aggregated over 4 strips
                    # [c, w, h-chunk] layout so the DRAM write streams h-runs
                    y2m = p1_out.tile([COUT, W, 64], bf16, tag="y2m")
                    for ss in range(4):
                        s = sg * 4 + ss
                        h0 = s * STRIP1
                        xt = p1_in.tile([128, STRIP1 + 2, W + 2], bf16, tag="xt")
                        nc.sync.dma_start(out=xt[0:64], in_=xm_d[:, h0:h0 + 18, :])
                        nc.sync.dma_start(out=xt[64:128, 0:17, :],
                                          in_=xm_d[:, h0 + 1:h0 + 18, :])
                        rb = p1_in.tile([128, STRIP1, W], bf16, tag="rb")
                        r_src = bass.AP(
                            tensor=r_d.tensor if hasattr(r_d, "tensor") else r_d,
                            offset=h0 * W,
                            ap=[[0, 128], [W, STRIP1], [1, W]],
                        )
                        nc.gpsimd.dma_start(out=rb, in_=r_src)
                        st = p1_in.tile([1, STRIP1, W], bf16, tag="st")
                        nc.sync.dma_start(out=st, in_=s_d[h0:h0 + STRIP1, :])

                        for t in range(STRIP1 // 2):
                            r = 2 * t
                            q = psq.tile([COUT, 512], f32, tag="q")
                            for dx in range(3):
                                nc.tensor.matmul(
                                    q, wp_sb[:, dx * COUT:(dx + 1) * COUT],
                                    xt[:, r:r + 2, dx:dx + W],
                                    start=(dx == 0), stop=False)
                            for dx in range(3):
                                nc.tensor.matmul(
                                    q, ws_sb[:, dx * COUT:(dx + 1) * COUT],
                                    xt[0:64, r + 2:r + 4, dx:dx + W],
                                    start=False, stop=False)
                            nc.tensor.matmul(q, bias_sb, st[:, r:r + 2, :],
                                             start=False, stop=True)

                            y1 = p1_tmp.tile([COUT, 512], bf16, tag="y1")
                            nc.vector.tensor_tensor(
                                out=y1, in0=q, in1=rb[:, r:r + 2, :], op=OP.mult)
                            sq = p1_tmp.tile([COUT, 512], bf16, tag="sq")
                            sq_eng = nc.gpsimd if dbg_gp else nc.vector
                            sq_eng.tensor_tensor(out=sq, in0=y1, in1=y1,
                                                 op=OP.mult)
                            g = psg.tile([COUT, 512], f32, tag="g")
                            nc.tensor.matmul(g, gT_sb, sq, start=True, stop=True)
                            inv = p1_tmp.tile([COUT, 512], bf16, tag="inv")
                            nc.scalar.activation(out=inv, in_=g,
                                                 func=AF.Abs_reciprocal_sqrt,
                                                 bias=beta_sb, scale=1.0)
                            hpos = ss * 16 + r
                            y2m_slice = y2m[:, :, hpos:hpos + 2].rearrange(
                                "c w h -> c h w")
                            nc.vector.tensor_tensor(
                                out=y2m_slice,
                                in0=y1.rearrange("c (h w) -> c h w", h=2),
                                in1=inv.rearrange("c (h w) -> c h w", h=2),
                                op=OP.mult)

                    # write 64 rows to y2_d [w, c, h]: partition c, h runs of 64
                    hh0 = sg * 64
                    y2_dst = bass.AP(
                        tensor=y2_d.tensor if hasattr(y2_d, "tensor") else y2_d,
                        offset=hh0,
                        ap=[[H, COUT], [COUT * H, W], [1, 64]],
                    )
                    if dbg_scatter:
                        nc.sync.dma_start(out=y2_dst, in_=y2m)
                    else:
                        nc.sync.dma_start(
                            out=y2_d[0:COUT, :, hh0:hh0 + 64].rearrange(
                                "c w h -> c w h"),
                            in_=y2m)

            # ================= Phase 2: DWT via matmuls =================
            with (
                tc.tile_pool(name="p2_in", bufs=2) as p2_in,
                tc.tile_pool(name="p2_mid", bufs=2) as p2_mid,
                tc.tile_pool(name="p2_out", bufs=2) as p2_out,
                tc.tile_pool(name="psw", bufs=4, space="PSUM") as psw,
                tc.tile_pool(name="psh", bufs=3, space="PSUM") as psh,
            ):
                for g in range(NG if dbg_p2 else 0):
                    c0 = g * CG
                    y2w = []
                    for wh in range(2):
                        t_ = p2_in.tile([128, CG, H], bf16, tag=f"y2w{wh}")
                        nc.sync.dma_start(
                            out=t_, in_=y2_d[wh * 128:(wh + 1) * 128,
                                             c0:c0 + CG, :])
                        y2w.append(t_)

                    # W-stage: stationary = y2w[wh][:, ci, hb*128:+128] (w x h),
                    # moving = fw_sb[:, wh, :] -> psum[h, (lo|hi) wo] packed 2ci
                    lohi = []
                    for hb in range(2):
                        lh_sb = p2_mid.tile([128, CG, 256], bf16, tag=f"lohi{hb}")
                        lohi.append(lh_sb)
                    for hb in range(2):
                        for cp in range(CG // 2):
                            pw = psw.tile([128, 512], f32, tag="pw")
                            for ci in (2 * cp, 2 * cp + 1):
                                sl = pw[:, (ci % 2) * 256:(ci % 2) * 256 + 256]
                                for wh in range(2):
                                    nc.tensor.matmul(
                                        sl,
                                        y2w[wh][:, ci, hb * 128:hb * 128 + 128],
                                        fw_sb[:, wh, :],
                                        start=(wh == 0), stop=(wh == 1))
                            nc.vector.tensor_copy(
                                out=lohi[hb][:, 2 * cp:2 * cp + 2, :],
                                in_=pw.rearrange("p (c x) -> p c x", c=2))

                    # H-stage: ll/lh/hl/hh[ho, ci, wo]
                    # ll: filt0 on lo cols; lh: filt1 on lo; hl: filt0 on hi...
                    # out order (ll, lh, hl, hh); ll = H-lo(F0h) + W-lo cols,
                    # lh = H-lo + W-hi cols, hl = H-hi(F1h) + W-lo, hh = H-hi+W-hi
                    for oi, (fi, side) in enumerate(
                            ((0, 0), (0, 1), (1, 0), (1, 1))[:0 if dbg_p2stage < 2 else 4]):
                        osb = p2_out.tile([128, CG, WO], f32, tag=f"osb{oi}")
                        for nt in range(2):
                            ph = psh.tile([128, 512], f32, tag="ph")
                            for hb in range(2):
                                rhs = lohi[hb][:, nt * 4:nt * 4 + 4,
                                               side * 128:side * 128 + 128]
                                nc.tensor.matmul(
                                    ph, fh_sb[:, fi * 2 + hb, :], rhs,
                                    start=(hb == 0), stop=(hb == 1))
                            nc.scalar.copy(out=osb[:, nt * 4:nt * 4 + 4, :],
                                           in_=ph.rearrange("p (c x) -> p c x",
                                                            c=4))
                        if dbg_p2stage >= 3:
                            out_eng = {"sync": nc.sync, "gpsimd": nc.gpsimd,
                                       "scalar": nc.scalar}[
                                os.environ.get("BK_OUTDMA", "sync")]
                            out_eng.dma_start(
                                out=out_d[oi, :, c0:c0 + CG, :], in_=osb)

    nc.finalize()
    return nc


def _host_prep(tensor, mask, weight, bias, u, beta, gamma):
    f32 = np.float32
    O = weight.shape[0]
    wm = weight.reshape(O, -1).astype(f32)
    v = wm.T @ u.astype(f32)
    v = v / (np.linalg.norm(v) + 1e-12)
    sigma = np.linalg.norm(wm @ v)
    w_sn = (weight / sigma).astype(f32)

    m = mask[:, 0].astype(f32)  # [B,H,W]
    mp = np.pad(m, ((0, 0), (1, 1), (1, 1)))
    msum = np.zeros_like(m)
    for dy in range(3):
        for dx in range(3):
            msum += mp[:, dy:dy + H, dx:dx + W]
    valid = msum > 0
    R = np.where(valid, 9.0 / np.maximum(msum, 1e-8), 0.0).astype(f32)
    S = np.where(valid, msum / 9.0, 0.0).astype(f32)
    mval = valid.astype(f32)

    xm = (tensor * mask).astype(f32)
    xm_pad = np.zeros((B, CIN, H + 2, W + 2), BF16)
    xm_pad[:, :, 1:-1, 1:-1] = xm

    # conv weights: wp[dx] [128(c,dy01),128o], ws[dx] [64c,128o]
    wp = np.zeros((128, 3 * COUT), f32)
    ws = np.zeros((CIN, 3 * COUT), f32)
    for dx in range(3):
        wp[0:64, dx * COUT:(dx + 1) * COUT] = w_sn[:, :, 0, dx].T
        wp[64:128, dx * COUT:(dx + 1) * COUT] = w_sn[:, :, 1, dx].T
        ws[:, dx * COUT:(dx + 1) * COUT] = w_sn[:, :, 2, dx].T
    biasT = np.ascontiguousarray(bias.astype(f32).reshape(1, COUT))
    gammaT = np.ascontiguousarray(gamma.astype(f32).T)
    betac = np.ascontiguousarray(beta.astype(f32).reshape(COUT, 1))

    # DWT folded filter matrices
    F0 = _fold_filter(H0, W)   # [128, 256]
    F1 = _fold_filter(H1, W)
    fw = np.zeros((2, 128, 256), f32)
    for wh in range(2):
        fw[wh, :, 0:128] = F0[:, wh * 128:(wh + 1) * 128].T
        fw[wh, :, 128:256] = F1[:, wh * 128:(wh + 1) * 128].T
    fh = np.zeros((4, 128, 128), f32)
    for fi, F in enumerate((F0, F1)):
        for hb in range(2):
            fh[fi * 2 + hb] = F[:, hb * 128:(hb + 1) * 128].T

    return (w_sn, xm_pad, R.astype(BF16), S.astype(BF16), mval,
            wp.astype(BF16), ws.astype(BF16), biasT.astype(BF16),
            gammaT.astype(BF16), betac, fw.astype(BF16), fh.astype(BF16))


def _mask_down(m, K, axis):
    p = (K - 1) // 2
    pads = [(0, 0)] * m.ndim
    pads[axis] = (p, p)
    mp = np.pad(m, pads, mode='reflect')
    n_out = m.shape[axis] // 2
    shp = list(m.shape)
    shp[axis] = n_out
    out = np.zeros(shp, m.dtype)
    idx = [slice(None)] * m.ndim
    for k in range(K):
        ik = list(idx)
        ik[axis] = slice(k, k + 2 * n_out, 2)
        out = out + mp[tuple(ik)]
    return (out > 0).astype(m.dtype)


def kernel(tensor, mask, weight, bias, u, beta, gamma):
    global _BUILT, LAST_RESULT
    from concourse.bass_utils import run_bass_kernel_spmd

    tensor = np.asarray(tensor, np.float32)
    mask = np.asarray(mask, np.float32)

    (w_sn, xm_pad, Rb, Sb, mval, wp, ws, biasT, gammaT, betac, fw, fh) = \
        _host_prep(tensor, mask, np.asarray(weight), np.asarray(bias),
                   np.asarray(u), np.asarray(beta), np.asarray(gamma))

    if _BUILT is None:
        _BUILT = build_nc()
    nc = _BUILT

    in_maps = []
    for b in range(N_CORES):
        in_maps.append({
            "xm": np.ascontiguousarray(xm_pad[b]),
            "rmap": np.ascontiguousarray(Rb[b]),
            "smap": np.ascontiguousarray(Sb[b]),
            "wp": wp, "ws": ws,
            "biasT": biasT,
            "gammaT": gammaT,
            "beta": betac,
            "fw": fw, "fh": fh,
        })

    trace = bool(int(os.environ.get("BASS_KERNEL_TRACE", "0")))
    res = run_bass_kernel_spmd(nc, in_maps, core_ids=list(range(N_CORES)),
                               trace=trace)
    LAST_RESULT = res

    # device out: [4, ho, c, wo] -> [c, ho, wo]
    def gather(oi):
        return np.stack([res.results[b]["out"][oi].transpose(1, 0, 2)
                         for b in range(N_CORES)])

    ll, lh, hl, hh = gather(0), gather(1), gather(2), gather(3)

    m4 = mval[:, None]
    m_lo = _mask_down(m4, 9, 2)
    m_hi = _mask_down(m4, 7, 2)
    m_ll = _mask_down(m_lo, 9, 3)
    m_lh = _mask_down(m_lo, 7, 3)
    m_hl = _mask_down(m_hi, 9, 3)
    m_hh = _mask_down(m_hi, 7, 3)

    return (ll.astype(np.float32), m_ll, lh.astype(np.float32),
            hl.astype(np.float32), hh.astype(np.float32), m_lh, m_hl, m_hh)
